# revision 1
# baseline (speedup 1.0000x reference)
"""Trainium2 Bass kernel for the DAMIC-style model:
embedding lookup -> 3x Conv1d(+ReLU+max-pool over tokens) -> BiLSTM over T -> sigmoid head.

Sharding: data-parallel over batch (B=32 -> 4 per core on 8 cores); weights
replicated; both LSTM directions computed per core on its own batch shard; the
host only reshapes/concats.

Pipeline: sentences are ordered (t, b) and conv runs in 8-timestep chunks
alternating from both ends of the sequence, so forward/reverse LSTM steps
interleave with conv on the PE as soon as their gate inputs are ready.
"""

import sys

sys.path.insert(0, "/opt/trn_rl_repo")

import numpy as np
import ml_dtypes

BF16 = ml_dtypes.bfloat16

VOCAB, EMB = 32000, 300
EMB_PAD = 384
NF = 100
FSIZES = (3, 4, 5)
NTAPS = 12
HID, OUT = 256, 32
B, T, L = 32, 64, 48
NCORES = 8
B_CORE = B // NCORES          # 4
S_CORE = B_CORE * T           # 256 sentences/core, ordered (t, b)
NTOK = S_CORE * L             # 12288
CH_T = 8                      # timesteps per conv chunk
NCH = T // CH_T               # 8 conv chunks
CHSENT = CH_T * B_CORE        # 32 sentences per chunk
CHTOK = CHSENT * L            # 1536 tokens per chunk
GCH = CHTOK // 128            # 12 gather chunks (128 tokens) per conv chunk
SENT_PER_PS = 8               # sentences per conv psum tile (= 2 timesteps)
PS_COLS = SENT_PER_PS * L     # 384
G4 = 4 * HID
CONV_ORDER = [0, 7, 1, 6, 2, 5, 3, 4]
PAIRS_OF = {3: [(0, 1)], 4: [(0, 1), (2, 3)], 5: [(0, 1), (2, 3)]}
SINGLES_OF = {3: [2], 4: [], 5: [4]}
PAIR_LIST = [(3, 0), (4, 0), (4, 2), (5, 0), (5, 2)]
PAIR_IDX = {p: i for i, p in enumerate(PAIR_LIST)}

_PROG = None


def build_program(debug=False):
    import concourse.bass as bass
    import concourse.tile as tile
    from concourse import bacc, mybir

    f32 = mybir.dt.float32
    bf16 = mybir.dt.bfloat16
    AF = mybir.ActivationFunctionType

    nc = bacc.Bacc("TRN2", target_bir_lowering=False, debug=False)

    idx_d = nc.dram_tensor("idx_w", [128, NTOK // 128], mybir.dt.int32, kind="ExternalInput").ap()
    iden_d = nc.dram_tensor("iden", [128, 128], bf16, kind="ExternalInput").ap()
    emb_d = nc.dram_tensor("emb_p", [VOCAB, EMB_PAD], bf16, kind="ExternalInput").ap()
    convw_d = nc.dram_tensor("convw", [128, 3, NTAPS * NF], bf16, kind="ExternalInput").ap()
    convw2_d = nc.dram_tensor("convw2", [128, 5 * NF], bf16, kind="ExternalInput").ap()
    convb_d = nc.dram_tensor("convb", [NF, 3], f32, kind="ExternalInput").ap()
    wih_d = nc.dram_tensor("wih", [NF, 3, 2, G4], bf16, kind="ExternalInput").ap()
    bih_d = nc.dram_tensor("bih", [2, 2, G4], bf16, kind="ExternalInput").ap()
    whh_d = nc.dram_tensor("whh", [128, 2, 2, 8, 128], bf16, kind="ExternalInput").ap()
    headw_d = nc.dram_tensor("headw", [128, 4, OUT], bf16, kind="ExternalInput").ap()
    headb_d = nc.dram_tensor("headb", [OUT, 1], f32, kind="ExternalInput").ap()
    out_d = nc.dram_tensor("out_t", [OUT, S_CORE], f32, kind="ExternalOutput").ap()
    if debug:
        dbg_feats_d = nc.dram_tensor("dbg_feats", [NF, 3, S_CORE], f32, kind="ExternalOutput").ap()
        dbg_xg_d = nc.dram_tensor("dbg_xg", [128, 2, T * 32], f32, kind="ExternalOutput").ap()
        dbg_h_d = nc.dram_tensor("dbg_h", [128, (T + 1) * 16], f32, kind="ExternalOutput").ap()

    tap_of = {3: 0, 4: 3, 5: 7}

    with tile.TileContext(nc) as tc:
        with (
            tc.tile_pool(name="const", bufs=1) as const,
            tc.tile_pool(name="gat", bufs=2) as gat,
            tc.tile_pool(name="gtok", bufs=4) as gtok,
            tc.tile_pool(name="small", bufs=3) as small,
            tc.tile_pool(name="cstate", bufs=2) as cstate,
            tc.tile_pool(name="cpsum", bufs=1, space="PSUM") as cpsum,
            tc.tile_pool(name="xpsum", bufs=2, space="PSUM") as xpsum,
            tc.tile_pool(name="gpsum", bufs=1, space="PSUM") as gpsum,
            tc.tile_pool(name="hpsum", bufs=1, space="PSUM") as hpsum,
        ):
            # gather-critical loads first (gpsimd queue feeds the gathers)
            idx_sb = const.tile([128, NTOK // 128], mybir.dt.int32)
            nc.gpsimd.dma_start(out=idx_sb[:], in_=idx_d[:])
            iden_sb = const.tile([128, 128], bf16)
            nc.gpsimd.dma_start(out=iden_sb[:], in_=iden_d[:])
            # weights on the HWDGE queue so they don't block gathers
            convw_sb = const.tile([128, 3, NTAPS * NF], bf16)
            nc.sync.dma_start(out=convw_sb[:], in_=convw_d[:])
            convw2_sb = const.tile([128, 5 * NF], bf16)
            nc.sync.dma_start(out=convw2_sb[:], in_=convw2_d[:])
            convb_sb = const.tile([NF, 3], f32)
            nc.sync.dma_start(out=convb_sb[:], in_=convb_d[:])
            wih_sb = const.tile([NF, 3, 2, G4], bf16)
            nc.sync.dma_start(out=wih_sb[:], in_=wih_d[:])
            bih_sb = const.tile([2, 2, G4], bf16)
            nc.sync.dma_start(out=bih_sb[:], in_=bih_d[:])
            whh_sb = const.tile([128, 2, 2, 8, 128], bf16)
            nc.sync.dma_start(out=whh_sb[:], in_=whh_d[:])
            headw_sb = const.tile([128, 4, OUT], bf16)
            nc.sync.dma_start(out=headw_sb[:], in_=headw_d[:])
            headb_sb = const.tile([OUT, 1], f32)
            nc.sync.dma_start(out=headb_sb[:], in_=headb_d[:])

            ones_sb = const.tile([2, S_CORE], bf16)
            nc.vector.memset(ones_sb[:], 1.0)

            feats = [const.tile([NF, S_CORE], bf16, tag=f"f{fs}", name=f"f{fs}") for fs in FSIZES]
            # xgT: [128, (d, t, g, b)] fp32 — both dirs in one tile
            xgT = const.tile([128, 2, T, 8, B_CORE], f32)
            # hseq: [128, slot, dir, ktile, b] bf16; slot 0 = h0 = 0
            hseq = const.tile([128, T + 1, 2, 2, B_CORE], bf16)
            nc.vector.memset(hseq[:, 0], 0.0)
            c_prev = []
            for d in range(2):
                c0 = cstate.tile([128, 2, B_CORE], f32, tag=f"c{d}", name=f"c0{d}")
                nc.vector.memset(c0[:], 0.0)
                c_prev.append(c0)

            def conv_chunk(sc):
                g = gat.tile([128, 3, CHTOK], bf16, tag="g", name="g")
                for c in range(GCH):
                    gc = GCH * sc + c
                    gt = gtok.tile([128, EMB_PAD], bf16, tag="gt", name="gt")
                    nc.gpsimd.indirect_dma_start(
                        out=gt[:], out_offset=None, in_=emb_d[:],
                        in_offset=bass.IndirectOffsetOnAxis(
                            ap=idx_sb[:, gc : gc + 1], axis=0
                        ),
                    )
                    for e in range(3):
                        nc.sync.dma_start_transpose(
                            g[:, e, 128 * c : 128 * (c + 1)],
                            gt[:, 128 * e : 128 * (e + 1)],
                        )
                # tap-tail pairing: rows 64:108 of block 2 = rows 0:44 shifted by 1 token
                nc.gpsimd.dma_start(
                    out=g[64:108, 2, 0 : CHTOK - 1], in_=g[0:44, 2, 1:CHTOK]
                )
                for j in range(CHSENT // SENT_PER_PS):  # 4 psum tiles
                    base = PS_COLS * j
                    s0 = CHSENT * sc + SENT_PER_PS * j
                    for fi, fs in enumerate(FSIZES):
                        ps = cpsum.tile([NF, PS_COLS], f32, tag=f"ps{fs}", name=f"ps{fs}")
                        mms = []
                        for kk in range(2):
                            for k in range(fs):
                                ti = tap_of[fs] + k
                                mms.append((convw_sb[:, kk, ti * NF : (ti + 1) * NF], kk, k, 0))
                        for (pa, pb) in PAIRS_OF[fs]:
                            pi = PAIR_IDX[(fs, pa)]
                            mms.append((convw2_sb[:, pi * NF : (pi + 1) * NF], 2, pa, 1))
                        for k in SINGLES_OF[fs]:
                            ti = tap_of[fs] + k
                            mms.append((convw_sb[:, 2, ti * NF : (ti + 1) * NF], 2, k, 0))
                        for mm, (lhsT, kk, k, is_pair) in enumerate(mms):
                            n = min(PS_COLS, CHTOK - is_pair - base - k)
                            nc.tensor.matmul(
                                ps[:, 0:n],
                                lhsT,
                                g[:, kk, base + k : base + k + n],
                                start=(mm == 0),
                                stop=(mm == len(mms) - 1),
                            )
                        ps3 = ps[:].rearrange("p (s l) -> p s l", l=L)
                        nc.vector.reduce_max(
                            out=feats[fi][:, s0 : s0 + SENT_PER_PS],
                            in_=ps3[:, :, 0 : L - fs + 1],
                            axis=mybir.AxisListType.X,
                        )
                for fi in range(3):
                    sl = slice(CHSENT * sc, CHSENT * (sc + 1))
                    nc.scalar.activation(
                        out=feats[fi][:, sl], in_=feats[fi][:, sl], func=AF.Relu,
                        bias=convb_sb[:, fi : fi + 1],
                    )

            def xg_chunk(sc):
                cols = slice(CHSENT * sc, CHSENT * (sc + 1))  # feats cols (t,b)
                for d in range(2):
                    for gt in range(8):
                        ps = xpsum.tile([128, CHSENT], f32, tag="xp", name="xp")
                        for kk in range(3):
                            nc.tensor.matmul(
                                ps[:],
                                wih_sb[:, kk, d, 128 * gt : 128 * (gt + 1)],
                                feats[kk][:, cols],
                                start=(kk == 0),
                                stop=False,
                            )
                        nc.tensor.matmul(
                            ps[:],
                            bih_sb[:, d, 128 * gt : 128 * (gt + 1)],
                            ones_sb[:, cols],
                            start=False,
                            stop=True,
                        )
                        # psum cols (t,b) -> xgT[:, d, t, gt, b]
                        nc.scalar.copy(
                            out=xgT[:, d, CH_T * sc : CH_T * (sc + 1), gt, :],
                            in_=ps[:].rearrange("p (t b) -> p t b", b=B_CORE),
                        )

            def lstm_step(s):
                # fwd(t=s) and rev(tt=T-1-s) emitted as two independent chains
                for d in range(2):
                    tt = s if d == 0 else T - 1 - s
                    rslot = (s if d == 0 else (T + 1 - s)) if s > 0 else 0
                    ps = gpsum.tile([128, 32], f32, tag=f"gp{d}", name=f"gp{d}")
                    for gt in range(8):
                        for kk in range(2):
                            nc.tensor.matmul(
                                ps[:, 4 * gt : 4 * gt + 4],
                                whh_sb[:, d, kk, gt, :],
                                hseq[:, rslot, d, kk, :],
                                start=(kk == 0),
                                stop=(kk == 1),
                            )
                    gates = small.tile([128, 32], f32, tag=f"gates{d}", name=f"gates{d}")
                    nc.vector.tensor_add(
                        gates[:],
                        ps[:],
                        xgT[:, d, tt, :, :].rearrange("p g b -> p (g b)"),
                    )
                    sig = small.tile([128, 24], f32, tag=f"sig{d}", name=f"sig{d}")
                    nc.scalar.activation(sig[:], gates[:, 0:24], AF.Sigmoid)
                    tg = small.tile([128, 8], f32, tag=f"tg{d}", name=f"tg{d}")
                    nc.scalar.activation(tg[:], gates[:, 24:32], AF.Tanh)
                    t1 = small.tile([128, 8], f32, tag=f"t1{d}", name=f"t1{d}")
                    nc.vector.tensor_mul(t1[:], sig[:, 0:8], tg[:])
                    cn = cstate.tile([128, 2, B_CORE], f32, tag=f"c{d}", name=f"c{d}")
                    nc.vector.tensor_mul(cn[:], sig[:, 8:16], c_prev[d][:])
                    nc.vector.tensor_add(cn[:], cn[:], t1[:])
                    c_prev[d] = cn
                    thc = small.tile([128, 8], f32, tag=f"thc{d}", name=f"thc{d}")
                    nc.scalar.activation(thc[:], cn[:], AF.Tanh)
                    nc.vector.tensor_mul(hseq[:, tt + 1, d], sig[:, 16:24], thc[:])

            done = set()
            state = {"emitted": 0}

            def ready_steps():
                while state["emitted"] < T:
                    s = state["emitted"]
                    if (s // CH_T) in done and ((T - 1 - s) // CH_T) in done:
                        lstm_step(s)
                        state["emitted"] += 1
                    else:
                        break

            for sc in CONV_ORDER:
                conv_chunk(sc)
                xg_chunk(sc)
                done.add(sc)
                ready_steps()
            assert state["emitted"] == T

            if debug:
                dbg_f = const.tile([NF, 3, S_CORE], f32)
                for fi in range(3):
                    nc.vector.tensor_copy(dbg_f[:, fi, :], feats[fi][:])
                nc.gpsimd.dma_start(out=dbg_feats_d[:], in_=dbg_f[:])
                xg_flat = xgT[:].rearrange("p d t g b -> p d (t g b)")
                nc.gpsimd.dma_start(out=dbg_xg_d[:], in_=xg_flat)
                hf32 = const.tile([128, (T + 1) * 16], f32)
                nc.vector.tensor_copy(hf32[:], hseq[:].rearrange("p a b c d -> p (a b c d)"))
                nc.gpsimd.dma_start(out=dbg_h_d[:], in_=hf32[:])

            # head: out.T[o, (b,t)] = sigmoid(head_w @ h2 + b)
            hp = hpsum.tile([OUT, S_CORE], f32)
            for qd in range(4):
                d, kk = qd // 2, qd % 2
                rhs = hseq[:, 1 : T + 1, d, kk, :].rearrange("p t b -> p b t")
                nc.tensor.matmul(
                    hp[:], headw_sb[:, qd, :], rhs, start=(qd == 0), stop=(qd == 3)
                )
            out_sb = small.tile([OUT, S_CORE], f32, tag="outsb", name="outsb")
            nc.scalar.activation(out_sb[:], hp[:], AF.Sigmoid, bias=headb_sb[:])
            nc.gpsimd.dma_start(out=out_d[:], in_=out_sb[:])

    nc.compile()
    return nc


def get_program():
    global _PROG
    if _PROG is None:
        _PROG = build_program()
    return _PROG


# ------------- host-side data prep (reshape/transpose/pad/cast only) -------------

def prep_shared(inputs):
    emb = np.zeros((VOCAB, EMB_PAD), np.float32)
    emb[:, :EMB] = inputs["emb"]
    emb_p = emb.astype(BF16)

    Wfull = np.zeros((EMB_PAD, NTAPS * NF), np.float32)
    col = 0
    for fs in FSIZES:
        w = np.asarray(inputs[f"conv_w{fs}"], np.float32)
        for k in range(fs):
            Wfull[:EMB, col : col + NF] = w[:, :, k].T
            col += NF
    convw = Wfull.reshape(3, 128, NTAPS * NF).transpose(1, 0, 2).astype(BF16)

    convb = np.stack(
        [np.asarray(inputs[f"conv_b{fs}"], np.float32) for fs in FSIZES], axis=1
    )

    convw2 = np.zeros((128, 5 * NF), np.float32)
    for i, (fs, ka) in enumerate(PAIR_LIST):
        w = np.asarray(inputs[f"conv_w{fs}"], np.float32)
        convw2[0:44, i * NF : (i + 1) * NF] = w[:, 256:300, ka].T
        convw2[64:108, i * NF : (i + 1) * NF] = w[:, 256:300, ka + 1].T

    perm = np.concatenate(
        [np.arange(0, 256), np.arange(256, 512), np.arange(768, 1024), np.arange(512, 768)]
    )  # i,f,g,o -> i,f,o,g

    wih_h = np.zeros((NF, 3, 2, G4), np.float32)
    bih_h = np.zeros((2, 2, G4), np.float32)
    whh_h = np.zeros((128, 2, 2, 8, 128), np.float32)
    for d, tag in ((0, "f"), (1, "r")):
        wih = np.asarray(inputs[f"w_ih_{tag}"], np.float32)[perm]
        whh = np.asarray(inputs[f"w_hh_{tag}"], np.float32)[perm]
        bih_h[0, d] = np.asarray(inputs[f"b_ih_{tag}"], np.float32)[perm]
        bih_h[1, d] = np.asarray(inputs[f"b_hh_{tag}"], np.float32)[perm]
        for kk in range(3):
            wih_h[:, kk, d, :] = wih[:, kk * NF : (kk + 1) * NF].T
        whh_h[:, d] = whh.reshape(8, 128, 2, 128).transpose(3, 2, 0, 1)

    headw = np.asarray(inputs["head_w"], np.float32)
    headw_h = headw.T.reshape(4, 128, OUT).transpose(1, 0, 2).astype(BF16)
    headb_h = np.asarray(inputs["head_b"], np.float32).reshape(OUT, 1)

    return {
        "emb_p": emb_p,
        "convw": np.ascontiguousarray(convw),
        "convw2": np.ascontiguousarray(convw2.astype(BF16)),
        "convb": np.ascontiguousarray(convb),
        "wih": wih_h.astype(BF16),
        "bih": bih_h.astype(BF16),
        "whh": np.ascontiguousarray(whh_h.astype(BF16)),
        "headw": np.ascontiguousarray(headw_h),
        "headb": headb_h,
        "iden": np.eye(128, dtype=BF16),
    }


def prep_core_idx(dialogue, core):
    """(t, b)-ordered token stream; token c*128+p at [p, c]."""
    dia = np.asarray(dialogue[B_CORE * core : B_CORE * (core + 1)], np.int32)
    ids = dia.transpose(1, 0, 2).reshape(-1)  # (t, b, l)
    return np.ascontiguousarray(ids.reshape(NTOK // 128, 128).T)


def kernel(**inputs):
    from concourse.bass_utils import run_bass_kernel_spmd

    nc = get_program()
    shared = prep_shared(inputs)
    dialogue = np.asarray(inputs["dialogue"])
    in_maps = []
    for core in range(NCORES):
        m = dict(shared)
        m["idx_w"] = prep_core_idx(dialogue, core)
        in_maps.append(m)
    res = run_bass_kernel_spmd(nc, in_maps, list(range(NCORES)))
    out = np.zeros((B, T, OUT), np.float32)
    for core in range(NCORES):
        o = res.results[core]["out_t"]  # [32, 256] col = b*64 + t
        out[B_CORE * core : B_CORE * (core + 1)] = o.reshape(OUT, B_CORE, T).transpose(
            1, 2, 0
        )
    return out



# revision 11
# speedup vs baseline: 1.4721x; 1.4721x over previous
"""Trainium2 Bass kernel for the DAMIC-style model:
embedding lookup -> 3x Conv1d(+ReLU+max-pool over tokens) -> BiLSTM over T -> sigmoid head.

Sharding: data-parallel over batch (B=32 -> 4 per core on 8 cores); weights
replicated; both LSTM directions computed per core on its own batch shard.

v2: fp8 datapath. Embedding table stored fp8 (304B rows); gathers move half
the bytes; transposes move 2-byte *pairs* of fp8 values so the conv rhs comes
out pair-interleaved, which is exactly the DoubleRow ifmap format -> conv
matmuls pair adjacent emb dims (K=256/instr, ~2x PE throughput). Leftover dims
256..299 are handled by one packed DR matmul per (fs, col-group) using
token-shifted row bands built with one SBUF DMA per chunk. LSTM whh runs fp8
(FWL weight loads); gate bias folds into the xg psum eviction; cell update
uses a merged (i*g | f*c) DVE multiply. Transposes split across the sync and
scalar HWDGE queues; the gpsimd queue carries only gathers so it never
head-of-line blocks.
"""

import sys

sys.path.insert(0, "/opt/trn_rl_repo")

import numpy as np
import ml_dtypes

BF16 = ml_dtypes.bfloat16
F8 = ml_dtypes.float8_e4m3

VOCAB, EMB = 32000, 300
ROW = 304                     # fp8 bytes per emb row (300 used, 16B aligned)
NF = 100
FSIZES = (3, 4, 5)
TAPS = [(3, 0), (3, 1), (3, 2),
        (4, 0), (4, 1), (4, 2), (4, 3),
        (5, 0), (5, 1), (5, 2), (5, 3), (5, 4)]
FSI = {3: 0, 4: 1, 5: 2}
HID, OUT = 256, 32
B, T, L = 32, 64, 48
NCORES = 8
B_CORE = B // NCORES          # 4
S_CORE = B_CORE * T           # 256 sentences/core, ordered (t, b)
NTOK = S_CORE * L             # 12288
CH_T = 8                      # timesteps per conv chunk
NCH = T // CH_T               # 8 conv chunks
CHSENT = CH_T * B_CORE        # 32 sentences per chunk
CHTOK = CHSENT * L            # 1536 tokens per chunk
GCH = CHTOK // 128            # 12 gather chunks (128 tokens) per conv chunk
PS_COLS = 8 * L               # 384 cols per conv psum tile (8 sentences)
CONV_ORDER = [0, 7, 1, 6, 2, 5, 3, 4]

_PROG = None


def build_program():
    import concourse.bass as bass
    import concourse.tile as tile
    from concourse import bacc, mybir

    import bass_rust

    f32 = mybir.dt.float32
    bf16 = mybir.dt.bfloat16
    fp8 = mybir.dt.float8e4
    AF = mybir.ActivationFunctionType
    DR = mybir.MatmulPerfMode.DoubleRow

    def strided(view, pairs):
        """Copy of `view` with free dims replaced by explicit [stride, count] pairs."""
        v = view.copy()
        v.ap = bass_rust.VecI64Pair([list(v.ap[0])] + [list(p) for p in pairs])
        return v

    nc = bacc.Bacc("TRN2", target_bir_lowering=False, debug=False)

    idx_d = nc.dram_tensor("idx_w", [128, NTOK // 128], mybir.dt.int32, kind="ExternalInput").ap()
    iden_d = nc.dram_tensor("iden", [128, 128], bf16, kind="ExternalInput").ap()
    emb_d = nc.dram_tensor("emb_p", [VOCAB, ROW], fp8, kind="ExternalInput").ap()
    wmain_d = nc.dram_tensor("wmain", [128, 12, 2, 128], fp8, kind="ExternalInput").ap()
    wleft_d = nc.dram_tensor("wleft", [110, 3, 2, 128], fp8, kind="ExternalInput").ap()
    convb_d = nc.dram_tensor("convb", [NF, 3], f32, kind="ExternalInput").ap()
    wih_d = nc.dram_tensor("wih", [NF, 3, 2, 1024], bf16, kind="ExternalInput").ap()
    bihp_d = nc.dram_tensor("bihp", [128, 2, 8], f32, kind="ExternalInput").ap()
    whh_d = nc.dram_tensor("whh", [128, 2, 2, 8, 128], fp8, kind="ExternalInput").ap()
    headw_d = nc.dram_tensor("headw", [128, 4, OUT], fp8, kind="ExternalInput").ap()
    headb_d = nc.dram_tensor("headb", [OUT, 1], f32, kind="ExternalInput").ap()
    out_d = nc.dram_tensor("out_t", [OUT, S_CORE], f32, kind="ExternalOutput").ap()

    with tile.TileContext(nc) as tc:
        with (
            tc.tile_pool(name="const", bufs=1) as const,
            tc.tile_pool(name="gat", bufs=2) as gat,
            tc.tile_pool(name="gtok", bufs=4) as gtok,
            tc.tile_pool(name="small", bufs=3) as small,
            tc.tile_pool(name="cell", bufs=2) as cellp,
            tc.tile_pool(name="tps", bufs=1, space="PSUM") as tps,
            tc.tile_pool(name="cpsum", bufs=1, space="PSUM") as cpsum,
            tc.tile_pool(name="xpsum", bufs=2, space="PSUM") as xpsum,
            tc.tile_pool(name="gpsum", bufs=1, space="PSUM") as gpsum,
            tc.tile_pool(name="hpsum", bufs=1, space="PSUM") as hpsum,
        ):
            idx_sb = const.tile([128, NTOK // 128], mybir.dt.int32)
            nc.gpsimd.dma_start(out=idx_sb[:], in_=idx_d[:])
            iden = const.tile([128, 128], bf16)
            nc.sync.dma_start(out=iden[:], in_=iden_d[:])
            wmain = const.tile([128, 12, 2, 128], fp8)
            nc.sync.dma_start(out=wmain[:], in_=wmain_d[:])
            wleft = const.tile([110, 3, 2, 128], fp8)
            nc.sync.dma_start(out=wleft[:], in_=wleft_d[:])
            convb = const.tile([NF, 3], f32)
            nc.sync.dma_start(out=convb[:], in_=convb_d[:])
            wih = const.tile([NF, 3, 2, 1024], bf16)
            nc.sync.dma_start(out=wih[:], in_=wih_d[:])
            bihp = const.tile([128, 2, 8], f32)
            nc.sync.dma_start(out=bihp[:], in_=bihp_d[:])
            whh = const.tile([128, 2, 2, 8, 128], fp8)
            nc.sync.dma_start(out=whh[:], in_=whh_d[:])
            headw = const.tile([128, 4, OUT], fp8)
            nc.sync.dma_start(out=headw[:], in_=headw_d[:])
            headb = const.tile([OUT, 1], f32)
            nc.sync.dma_start(out=headb[:], in_=headb_d[:])

            feats = [const.tile([NF, S_CORE], bf16, tag=f"f{fs}", name=f"f{fs}") for fs in FSIZES]
            # xgT[:, d, t, (gtype, kko, b)] bf16, bias folded in
            xgT = const.tile([128, 2, T, 32], bf16)
            # hseq[:, d, slot, kk, b] fp8; slot 0 = h0 = 0
            hseq = const.tile([128, 2, T + 1, 2, B_CORE], fp8)
            nc.vector.memset(hseq[:, :, 0], 0.0)
            # per-dir cell tiles [128, (tg|cp), kk, b]; cp of step0 = 0
            cprev = []
            for d in range(2):
                t0 = cellp.tile([128, 2, 2, B_CORE], f32, tag=f"tgcp{d}", name=f"tgcp{d}")
                nc.vector.memset(t0[:], 0.0)
                cprev.append(t0)
            # both dirs' LSTM gate psums share one bank (element-disjoint halves)
            gates = gpsum.tile([128, 2, 32], f32)

            def conv_chunk(sc):
                gmain = gat.tile([128, CHTOK], bf16, tag="gm", name="gm")
                gleft = gat.tile([110, CHTOK], bf16, tag="gl", name="gl")
                for c in range(GCH):
                    gc = GCH * sc + c
                    gt = gtok.tile([128, ROW], fp8, tag="gt", name="gt")
                    nc.gpsimd.indirect_dma_start(
                        out=gt[:], out_offset=None, in_=emb_d[:],
                        in_offset=bass.IndirectOffsetOnAxis(
                            ap=idx_sb[:, gc : gc + 1], axis=0
                        ),
                    )
                    gt16 = gt[:].bitcast(bf16)          # [128, 152]
                    q = nc.sync if c % 2 == 0 else nc.scalar
                    q.dma_start_transpose(
                        gmain[:, 128 * c : 128 * (c + 1)], gt16[:, 0:128]
                    )
                    tp = tps.tile([22, 128], bf16, tag="tp", name="tp")
                    nc.tensor.transpose(tp[:], gt16[:, 128:150], iden[:])
                    nc.scalar.copy(out=gleft[0:22, 128 * c : 128 * (c + 1)], in_=tp[:])
                # token-shift bands for leftover taps 1..4
                for j in range(1, 5):
                    nc.scalar.dma_start(
                        out=gleft[22 * j : 22 * (j + 1), 0 : CHTOK - j],
                        in_=gleft[0:22, j:CHTOK],
                    )
                gm8 = gmain[:].bitcast(fp8).rearrange("p (t k) -> p k t", k=2)
                gl8 = gleft[:].bitcast(fp8).rearrange("p (t k) -> p k t", k=2)
                for jp in range(4):
                    base = PS_COLS * jp
                    pst = {fs: cpsum.tile([128, PS_COLS], f32, tag=f"ps{fs}", name=f"ps{fs}")
                           for fs in FSIZES}
                    for ti, (fs, k) in enumerate(TAPS):
                        n = min(PS_COLS, CHTOK - base - k)
                        nc.tensor.matmul(
                            pst[fs][:, 0:n],
                            wmain[:, ti],
                            gm8[:, :, base + k : base + k + n],
                            start=(k == 0), stop=False, perf_mode=DR,
                        )
                    for fs in FSIZES:
                        nc.tensor.matmul(
                            pst[fs][:],
                            wleft[0 : 22 * fs, FSI[fs]],
                            gl8[0 : 22 * fs, :, base : base + PS_COLS],
                            start=False, stop=True, perf_mode=DR,
                        )
                    for fs in FSIZES:
                        s0 = CHSENT * sc + 8 * jp
                        ps3 = pst[fs][:].rearrange("p (s l) -> p s l", l=L)
                        nc.vector.reduce_max(
                            out=feats[FSI[fs]][:, s0 : s0 + 8],
                            in_=ps3[0:NF, :, 0 : L - fs + 1],
                            axis=mybir.AxisListType.X,
                        )
                for fi in range(3):
                    sl = slice(CHSENT * sc, CHSENT * (sc + 1))
                    nc.scalar.activation(
                        out=feats[fi][:, sl], in_=feats[fi][:, sl], func=AF.Relu,
                        bias=convb[:, fi : fi + 1],
                    )

            def xg_pair(sa, sb):
                # feats cols for the two chunks via a strided pair dim
                dcol = 32 * (sb - sa)
                dslot = (sb - sa) * 8 * 32
                for d in range(2):
                    for gt in range(8):
                        ps = xpsum.tile([128, 64], f32, tag="xp", name="xp")
                        for kk in range(3):
                            rhs = strided(
                                feats[kk][:, 32 * sa : 32 * sa + 32],
                                [[dcol, 2], [1, 32]],
                            )
                            nc.tensor.matmul(
                                ps[:],
                                wih[:, kk, d, 128 * gt : 128 * (gt + 1)],
                                rhs,
                                start=(kk == 0), stop=(kk == 2),
                            )
                        # evict with bias: psum cols (chunk, t, b) -> xgT slots
                        ov = strided(
                            xgT[:, d, 8 * sa, 4 * gt : 4 * gt + 4],
                            [[dslot, 2], [32, 8], [1, 4]],
                        )
                        nc.vector.tensor_scalar_add(
                            ov,
                            ps[:].rearrange("p (c t b) -> p c t b", c=2, b=B_CORE),
                            bihp[:, d, gt : gt + 1],
                        )

            def lstm_step(s):
                for d in range(2):
                    tt = s if d == 0 else T - 1 - s
                    rslot = (s if d == 0 else (T + 1 - s)) if s > 0 else 0
                    ps = gates[:, d]
                    nc.tensor.matmul(
                        ps, iden[:], xgT[:, d, tt], start=True, stop=False,
                        skip_group_check=True,
                    )
                    for gt in range(8):
                        for kk in range(2):
                            nc.tensor.matmul(
                                ps[:, 4 * gt : 4 * gt + 4],
                                whh[:, d, kk, gt, :],
                                hseq[:, d, rslot, kk, :],
                                start=False, stop=(gt == 7 and kk == 1),
                                skip_group_check=True,
                            )
                    # psum cols: (gtype i,f,o,g) x (kk) x (b)
                    sig = small.tile([128, 24], f32, tag=f"sig{d}", name=f"sig{d}")
                    nc.scalar.activation(sig[:], ps[:, 0:24], AF.Sigmoid)
                    tgcp = cprev[d]
                    nc.scalar.activation(tgcp[:, 0], ps[:, 24:32], AF.Tanh)
                    mo = small.tile([128, 16], f32, tag=f"mo{d}", name=f"mo{d}")
                    nc.vector.tensor_mul(
                        mo[:], sig[:, 0:16], tgcp[:].rearrange("p a b c -> p (a b c)")
                    )
                    tnext = cellp.tile([128, 2, 2, B_CORE], f32, tag=f"tgcp{d}", name=f"tgcp{d}")
                    nc.vector.tensor_add(tnext[:, 1], mo[:, 0:8], mo[:, 8:16])
                    cprev[d] = tnext
                    thc = small.tile([128, 8], f32, tag=f"thc{d}", name=f"thc{d}")
                    nc.scalar.activation(thc[:], tnext[:, 1], AF.Tanh)
                    nc.vector.tensor_mul(hseq[:, d, tt + 1], sig[:, 16:24], thc[:])

            done = set()
            state = {"emitted": 0}

            def ready_steps():
                while state["emitted"] < T:
                    s = state["emitted"]
                    if (s // CH_T) in done and ((T - 1 - s) // CH_T) in done:
                        lstm_step(s)
                        state["emitted"] += 1
                    else:
                        break

            conv_done = set()
            for sc in CONV_ORDER:
                conv_chunk(sc)
                conv_done.add(sc)
                if (7 - sc) in conv_done:
                    xg_pair(min(sc, 7 - sc), max(sc, 7 - sc))
                    done.add(sc)
                    done.add(7 - sc)
                    ready_steps()
            assert state["emitted"] == T, state["emitted"]

            # head: out.T[o, (b,t)] = sigmoid(head_w @ h2 + b)
            hp = hpsum.tile([OUT, S_CORE], f32)
            for qd in range(4):
                d, kk = qd // 2, qd % 2
                rhs = hseq[:, d, 1 : T + 1, kk, :].rearrange("p t b -> p b t")
                nc.tensor.matmul(
                    hp[:], headw[:, qd, :], rhs, start=(qd == 0), stop=(qd == 3)
                )
            out_sb = small.tile([OUT, S_CORE], f32, tag="outsb", name="outsb")
            nc.scalar.activation(out_sb[:], hp[:], AF.Sigmoid, bias=headb[:])
            nc.gpsimd.dma_start(out=out_d[:], in_=out_sb[:])

    nc.compile()
    return nc


def get_program():
    global _PROG
    if _PROG is None:
        _PROG = build_program()
    return _PROG


# ------------- host-side data prep (reshape/transpose/pad/cast only) -------------

def prep_shared(inputs):
    emb = np.zeros((VOCAB, ROW), np.float32)
    emb[:, :EMB] = inputs["emb"]
    emb_p = emb.astype(F8)

    # main conv weights: DR pairs over emb dims 0..255
    wmain = np.zeros((128, 12, 2, 128), np.float32)
    wleft = np.zeros((110, 3, 2, 128), np.float32)
    for ti, (fs, k) in enumerate(TAPS):
        w = np.asarray(inputs[f"conv_w{fs}"], np.float32)
        for p in range(128):
            wmain[p, ti, 0, 0:NF] = w[:, 2 * p, k]
            wmain[p, ti, 1, 0:NF] = w[:, 2 * p + 1, k]
    for fs in FSIZES:
        w = np.asarray(inputs[f"conv_w{fs}"], np.float32)
        for j in range(fs):
            for q in range(22):
                wleft[22 * j + q, FSI[fs], 0, 0:NF] = w[:, 256 + 2 * q, j]
                wleft[22 * j + q, FSI[fs], 1, 0:NF] = w[:, 256 + 2 * q + 1, j]

    convb = np.stack(
        [np.asarray(inputs[f"conv_b{fs}"], np.float32) for fs in FSIZES], axis=1
    )

    perm = np.concatenate(
        [np.arange(0, 256), np.arange(256, 512), np.arange(768, 1024), np.arange(512, 768)]
    )  # i,f,g,o -> i,f,o,g

    wih_h = np.zeros((NF, 3, 2, 1024), np.float32)
    bihp_h = np.zeros((128, 2, 8), np.float32)
    whh_h = np.zeros((128, 2, 2, 8, 128), np.float32)
    for d, tag in ((0, "f"), (1, "r")):
        wihm = np.asarray(inputs[f"w_ih_{tag}"], np.float32)[perm]
        whhm = np.asarray(inputs[f"w_hh_{tag}"], np.float32)[perm]
        bsum = (np.asarray(inputs[f"b_ih_{tag}"], np.float32)
                + np.asarray(inputs[f"b_hh_{tag}"], np.float32))[perm]
        for kk in range(3):
            wih_h[:, kk, d, :] = wihm[:, kk * NF : (kk + 1) * NF].T
        bihp_h[:, d, :] = bsum.reshape(8, 128).T
        whh_h[:, d] = whhm.reshape(8, 128, 2, 128).transpose(3, 2, 0, 1)

    headw = np.asarray(inputs["head_w"], np.float32)
    headw_h = headw.T.reshape(4, 128, OUT).transpose(1, 0, 2)
    headb_h = np.asarray(inputs["head_b"], np.float32).reshape(OUT, 1)

    return {
        "emb_p": emb_p,
        "iden": np.eye(128, dtype=BF16),
        "wmain": np.ascontiguousarray(wmain.astype(F8)),
        "wleft": np.ascontiguousarray(wleft.astype(F8)),
        "convb": np.ascontiguousarray(convb),
        "wih": np.ascontiguousarray(wih_h.astype(BF16)),
        "bihp": np.ascontiguousarray(bihp_h),
        "whh": np.ascontiguousarray(whh_h.astype(F8)),
        "headw": np.ascontiguousarray(headw_h.astype(F8)),
        "headb": headb_h,
    }


def prep_core_idx(dialogue, core):
    """(t, b)-ordered token stream; token c*128+p at [p, c]."""
    dia = np.asarray(dialogue[B_CORE * core : B_CORE * (core + 1)], np.int32)
    ids = dia.transpose(1, 0, 2).reshape(-1)  # (t, b, l)
    return np.ascontiguousarray(ids.reshape(NTOK // 128, 128).T)


def kernel(**inputs):
    from concourse.bass_utils import run_bass_kernel_spmd

    nc = get_program()
    shared = prep_shared(inputs)
    dialogue = np.asarray(inputs["dialogue"])
    in_maps = []
    for core in range(NCORES):
        m = dict(shared)
        m["idx_w"] = prep_core_idx(dialogue, core)
        in_maps.append(m)
    res = run_bass_kernel_spmd(nc, in_maps, list(range(NCORES)))
    out = np.zeros((B, T, OUT), np.float32)
    for core in range(NCORES):
        o = res.results[core]["out_t"]  # [32, 256] col = b*64 + t
        out[B_CORE * core : B_CORE * (core + 1)] = o.reshape(OUT, B_CORE, T).transpose(
            1, 2, 0
        )
    return out


# revision 16
# speedup vs baseline: 1.6209x; 1.1010x over previous
"""Trainium2 Bass kernel for the DAMIC-style model:
embedding lookup -> 3x Conv1d(+ReLU+max-pool over tokens) -> BiLSTM over T -> sigmoid head.

Sharding: data-parallel over batch (B=32 -> 4 per core on 8 cores); weights
replicated; both LSTM directions computed per core on its own batch shard.

v2: fp8 datapath. Embedding table stored fp8 (304B rows); gathers move half
the bytes; transposes move 2-byte *pairs* of fp8 values so the conv rhs comes
out pair-interleaved, which is exactly the DoubleRow ifmap format -> conv
matmuls pair adjacent emb dims (K=256/instr, ~2x PE throughput). Leftover dims
256..299 are handled by one packed DR matmul per (fs, col-group) using
token-shifted row bands built with one SBUF DMA per chunk. LSTM whh runs fp8
(FWL weight loads); gate bias folds into the xg psum eviction; cell update
uses a merged (i*g | f*c) DVE multiply. Transposes split across the sync and
scalar HWDGE queues; the gpsimd queue carries only gathers so it never
head-of-line blocks.
"""

import sys

sys.path.insert(0, "/opt/trn_rl_repo")

import numpy as np
import ml_dtypes

BF16 = ml_dtypes.bfloat16
F8 = ml_dtypes.float8_e4m3

VOCAB, EMB = 32000, 300
ROW = 304                     # fp8 bytes per emb row (300 used, 16B aligned)
NF = 100
FSIZES = (3, 4, 5)
TAPS = [(3, 0), (3, 1), (3, 2),
        (4, 0), (4, 1), (4, 2), (4, 3),
        (5, 0), (5, 1), (5, 2), (5, 3), (5, 4)]
FSI = {3: 0, 4: 1, 5: 2}
HID, OUT = 256, 32
B, T, L = 32, 64, 48
NCORES = 8
B_CORE = B // NCORES          # 4
S_CORE = B_CORE * T           # 256 sentences/core, ordered (t, b)
NTOK = S_CORE * L             # 12288
CH_T = 8                      # timesteps per conv chunk
NCH = T // CH_T               # 8 conv chunks
CHSENT = CH_T * B_CORE        # 32 sentences per chunk
CHTOK = CHSENT * L            # 1536 tokens per chunk
GCH = CHTOK // 128            # 12 gather chunks (128 tokens) per conv chunk
PS_COLS = 8 * L               # 384 cols per conv psum tile (8 sentences)
CONV_ORDER = [0, 7, 1, 6, 2, 5, 3, 4]

_PROG = None


def build_program():
    import concourse.bass as bass
    import concourse.tile as tile
    from concourse import bacc, mybir

    import bass_rust

    f32 = mybir.dt.float32
    bf16 = mybir.dt.bfloat16
    fp8 = mybir.dt.float8e4
    AF = mybir.ActivationFunctionType
    DR = mybir.MatmulPerfMode.DoubleRow

    def strided(view, pairs):
        """Copy of `view` with free dims replaced by explicit [stride, count] pairs."""
        v = view.copy()
        v.ap = bass_rust.VecI64Pair([list(v.ap[0])] + [list(p) for p in pairs])
        return v

    nc = bacc.Bacc("TRN2", target_bir_lowering=False, debug=False)

    idx_d = nc.dram_tensor("idx_w", [128, NTOK // 128], mybir.dt.int32, kind="ExternalInput").ap()
    iden_d = nc.dram_tensor("iden", [128, 128], bf16, kind="ExternalInput").ap()
    emb_d = nc.dram_tensor("emb_p", [VOCAB, ROW], fp8, kind="ExternalInput").ap()
    wmain_d = nc.dram_tensor("wmain", [128, 12, 2, 128], fp8, kind="ExternalInput").ap()
    wleft_d = nc.dram_tensor("wleft", [110, 3, 2, 128], fp8, kind="ExternalInput").ap()
    convb_d = nc.dram_tensor("convb", [NF, 3], f32, kind="ExternalInput").ap()
    wih_d = nc.dram_tensor("wih", [NF, 3, 2, 1024], bf16, kind="ExternalInput").ap()
    bihp_d = nc.dram_tensor("bihp", [128, 2, 8], f32, kind="ExternalInput").ap()
    whh_d = nc.dram_tensor("whh", [128, 2, 2, 8, 128], fp8, kind="ExternalInput").ap()
    headw_d = nc.dram_tensor("headw", [128, 4, OUT], fp8, kind="ExternalInput").ap()
    headb_d = nc.dram_tensor("headb", [OUT, 1], f32, kind="ExternalInput").ap()
    out_d = nc.dram_tensor("out_t", [OUT, S_CORE], f32, kind="ExternalOutput").ap()

    with tile.TileContext(nc) as tc:
        with (
            tc.tile_pool(name="const", bufs=1) as const,
            tc.tile_pool(name="gat", bufs=2) as gat,
            tc.tile_pool(name="gtok", bufs=16) as gtok,
            tc.tile_pool(name="small", bufs=3) as small,
            tc.tile_pool(name="cell", bufs=2) as cellp,
            tc.tile_pool(name="tps", bufs=1, space="PSUM") as tps,
            tc.tile_pool(name="cpsum", bufs=1, space="PSUM") as cpsum,
            tc.tile_pool(name="xpsum", bufs=2, space="PSUM") as xpsum,
            tc.tile_pool(name="gpsum", bufs=1, space="PSUM") as gpsum,
            tc.tile_pool(name="hpsum", bufs=1, space="PSUM") as hpsum,
        ):
            # conv-critical consts first; big LSTM/head consts deferred so they
            # don't block chunk-0 transposes on the sync queue
            idx_sb = const.tile([128, NTOK // 128], mybir.dt.int32)
            nc.gpsimd.dma_start(out=idx_sb[:], in_=idx_d[:])
            iden = const.tile([128, 128], bf16)
            nc.sync.dma_start(out=iden[:], in_=iden_d[:])
            wmain = const.tile([128, 12, 2, 128], fp8)
            nc.sync.dma_start(out=wmain[:], in_=wmain_d[:])
            wleft = const.tile([110, 3, 2, 128], fp8)
            nc.sync.dma_start(out=wleft[:], in_=wleft_d[:])
            convb = const.tile([NF, 3], f32)
            nc.sync.dma_start(out=convb[:], in_=convb_d[:])
            wih = const.tile([NF, 3, 2, 1024], bf16)
            bihp = const.tile([128, 2, 8], f32)
            whh = const.tile([128, 2, 2, 8, 128], fp8)
            headw = const.tile([128, 4, OUT], fp8)
            headb = const.tile([OUT, 1], f32)

            def load_lstm_consts():
                nc.scalar.dma_start(out=wih[:], in_=wih_d[:])
                nc.scalar.dma_start(out=bihp[:], in_=bihp_d[:])
                nc.scalar.dma_start(out=whh[:], in_=whh_d[:])
                nc.scalar.dma_start(out=headw[:], in_=headw_d[:])
                nc.scalar.dma_start(out=headb[:], in_=headb_d[:])

            feats = [const.tile([NF, S_CORE], bf16, tag=f"f{fs}", name=f"f{fs}") for fs in FSIZES]
            # xgT[:, d, t, (gtype, kko, b)] bf16, bias folded in
            xgT = const.tile([128, 2, T, 32], bf16)
            # hseq[:, d, slot, kk, b] fp8; slot 0 = h0 = 0
            hseq = const.tile([128, 2, T + 1, 2, B_CORE], fp8)
            nc.vector.memset(hseq[:, :, 0], 0.0)
            # per-dir cell tiles [128, (tg|cp), kk, b]; cp of step0 = 0
            cprev = []
            for d in range(2):
                t0 = cellp.tile([128, 2, 2, B_CORE], f32, tag=f"tgcp{d}", name=f"tgcp{d}")
                nc.vector.memset(t0[:], 0.0)
                cprev.append(t0)
            # both dirs' LSTM gate psums share one bank (element-disjoint halves)
            gates = gpsum.tile([128, 2, 32], f32)

            def conv_chunk(sc):
                gmain = gat.tile([128, CHTOK], bf16, tag="gm", name="gm")
                gleft = gat.tile([110, CHTOK], bf16, tag="gl", name="gl")
                for c in range(GCH):
                    gc = GCH * sc + c
                    gt = gtok.tile([128, ROW], fp8, tag="gt", name="gt")
                    nc.gpsimd.indirect_dma_start(
                        out=gt[:], out_offset=None, in_=emb_d[:],
                        in_offset=bass.IndirectOffsetOnAxis(
                            ap=idx_sb[:, gc : gc + 1], axis=0
                        ),
                    )
                    gt16 = gt[:].bitcast(bf16)          # [128, 152]
                    q = nc.sync if c % 2 == 0 else nc.scalar
                    q.dma_start_transpose(
                        gmain[:, 128 * c : 128 * (c + 1)], gt16[:, 0:128]
                    )
                    tp = tps.tile([22, 128], bf16, tag="tp", name="tp")
                    nc.tensor.transpose(tp[:], gt16[:, 128:150], iden[:])
                    nc.vector.tensor_copy(gleft[0:22, 128 * c : 128 * (c + 1)], tp[:])
                # token-shift bands for leftover taps 1..4
                for j in range(1, 5):
                    nc.sync.dma_start(
                        out=gleft[22 * j : 22 * (j + 1), 0 : CHTOK - j],
                        in_=gleft[0:22, j:CHTOK],
                    )
                gm8 = gmain[:].bitcast(fp8).rearrange("p (t k) -> p k t", k=2)
                gl8 = gleft[:].bitcast(fp8).rearrange("p (t k) -> p k t", k=2)
                for jp in range(4):
                    base = PS_COLS * jp
                    pst = {fs: cpsum.tile([128, PS_COLS], f32, tag=f"ps{fs}", name=f"ps{fs}")
                           for fs in FSIZES}
                    for ti, (fs, k) in enumerate(TAPS):
                        n = min(PS_COLS, CHTOK - base - k)
                        nc.tensor.matmul(
                            pst[fs][:, 0:n],
                            wmain[:, ti],
                            gm8[:, :, base + k : base + k + n],
                            start=(k == 0), stop=False, perf_mode=DR,
                        )
                    for fs in FSIZES:
                        nc.tensor.matmul(
                            pst[fs][:],
                            wleft[0 : 22 * fs, FSI[fs]],
                            gl8[0 : 22 * fs, :, base : base + PS_COLS],
                            start=False, stop=True, perf_mode=DR,
                        )
                    for fs in FSIZES:
                        s0 = CHSENT * sc + 8 * jp
                        ps3 = pst[fs][:].rearrange("p (s l) -> p s l", l=L)
                        nc.vector.reduce_max(
                            out=feats[FSI[fs]][:, s0 : s0 + 8],
                            in_=ps3[0:NF, :, 0 : L - fs + 1],
                            axis=mybir.AxisListType.X,
                        )
                for fi in range(3):
                    sl = slice(CHSENT * sc, CHSENT * (sc + 1))
                    nc.scalar.activation(
                        out=feats[fi][:, sl], in_=feats[fi][:, sl], func=AF.Relu,
                        bias=convb[:, fi : fi + 1],
                    )

            def xg_pair(sa, sb):
                # feats cols for the two chunks via a strided pair dim
                dcol = 32 * (sb - sa)
                dslot = (sb - sa) * 8 * 32
                for d in range(2):
                    for gt in range(8):
                        ps = xpsum.tile([128, 64], f32, tag="xp", name="xp")
                        for kk in range(3):
                            rhs = strided(
                                feats[kk][:, 32 * sa : 32 * sa + 32],
                                [[dcol, 2], [1, 32]],
                            )
                            nc.tensor.matmul(
                                ps[:],
                                wih[:, kk, d, 128 * gt : 128 * (gt + 1)],
                                rhs,
                                start=(kk == 0), stop=(kk == 2),
                            )
                        # evict with bias: psum cols (chunk, t, b) -> xgT slots
                        ov = strided(
                            xgT[:, d, 8 * sa, 4 * gt : 4 * gt + 4],
                            [[dslot, 2], [32, 8], [1, 4]],
                        )
                        nc.vector.tensor_scalar_add(
                            ov,
                            ps[:].rearrange("p (c t b) -> p c t b", c=2, b=B_CORE),
                            bihp[:, d, gt : gt + 1],
                        )

            def lstm_step(s, tail):
                for d in range(2):
                    tt = s if d == 0 else T - 1 - s
                    rslot = (s if d == 0 else (T + 1 - s)) if s > 0 else 0
                    ps = gates[:, d]
                    nc.tensor.matmul(
                        ps, iden[:], xgT[:, d, tt], start=True, stop=False,
                        skip_group_check=True,
                    )
                    for gt in range(8):
                        for kk in range(2):
                            nc.tensor.matmul(
                                ps[:, 4 * gt : 4 * gt + 4],
                                whh[:, d, kk, gt, :],
                                hseq[:, d, rslot, kk, :],
                                start=False, stop=(gt == 7 and kk == 1),
                                skip_group_check=True,
                            )
                    if tail:
                        # warm-keeper: harmless matmuls into the (idle) head psum
                        # keep the PE HAM at full clock through the LSTM tail
                        for u in range(3):
                            nc.tensor.matmul(
                                hp[:],
                                iden[:, 0:OUT],
                                xgT[:, d, 8 * ((3 * s + u) % 8) : 8 * ((3 * s + u) % 8) + 8]
                                .rearrange("p t g -> p (t g)"),
                                start=True, stop=True, skip_group_check=True,
                            )
                    # psum cols: (gtype i,f,o,g) x (kk) x (b)
                    sig = small.tile([128, 24], f32, tag=f"sig{d}", name=f"sig{d}")
                    nc.scalar.activation(sig[:], ps[:, 0:24], AF.Sigmoid)
                    tgcp = cprev[d]
                    nc.scalar.activation(tgcp[:, 0], ps[:, 24:32], AF.Tanh)
                    mo = small.tile([128, 16], f32, tag=f"mo{d}", name=f"mo{d}")
                    nc.vector.tensor_mul(
                        mo[:], sig[:, 0:16], tgcp[:].rearrange("p a b c -> p (a b c)")
                    )
                    tnext = cellp.tile([128, 2, 2, B_CORE], f32, tag=f"tgcp{d}", name=f"tgcp{d}")
                    nc.vector.tensor_add(tnext[:, 1], mo[:, 0:8], mo[:, 8:16])
                    cprev[d] = tnext
                    thc = small.tile([128, 8], f32, tag=f"thc{d}", name=f"thc{d}")
                    nc.scalar.activation(thc[:], tnext[:, 1], AF.Tanh)
                    nc.vector.tensor_mul(hseq[:, d, tt + 1], sig[:, 16:24], thc[:])

            done = set()
            state = {"emitted": 0}
            hp = hpsum.tile([OUT, S_CORE], f32)

            def ready_steps():
                tail = len(done) == NCH
                while state["emitted"] < T:
                    s = state["emitted"]
                    if (s // CH_T) in done and ((T - 1 - s) // CH_T) in done:
                        lstm_step(s, tail)
                        state["emitted"] += 1
                    else:
                        break

            conv_done = set()
            for i, sc in enumerate(CONV_ORDER):
                conv_chunk(sc)
                if i == 0:
                    load_lstm_consts()
                conv_done.add(sc)
                if (7 - sc) in conv_done:
                    xg_pair(min(sc, 7 - sc), max(sc, 7 - sc))
                    done.add(sc)
                    done.add(7 - sc)
                    ready_steps()
            assert state["emitted"] == T, state["emitted"]

            # head: out.T[o, (b,t)] = sigmoid(head_w @ h2 + b)
            for qd in range(4):
                d, kk = qd // 2, qd % 2
                rhs = hseq[:, d, 1 : T + 1, kk, :].rearrange("p t b -> p b t")
                nc.tensor.matmul(
                    hp[:], headw[:, qd, :], rhs, start=(qd == 0), stop=(qd == 3)
                )
            out_sb = small.tile([OUT, S_CORE], f32, tag="outsb", name="outsb")
            nc.scalar.activation(out_sb[:], hp[:], AF.Sigmoid, bias=headb[:])
            nc.gpsimd.dma_start(out=out_d[:], in_=out_sb[:])

    nc.compile()
    return nc


def get_program():
    global _PROG
    if _PROG is None:
        _PROG = build_program()
    return _PROG


# ------------- host-side data prep (reshape/transpose/pad/cast only) -------------

def prep_shared(inputs):
    emb = np.zeros((VOCAB, ROW), np.float32)
    emb[:, :EMB] = inputs["emb"]
    emb_p = emb.astype(F8)

    # main conv weights: DR pairs over emb dims 0..255
    wmain = np.zeros((128, 12, 2, 128), np.float32)
    wleft = np.zeros((110, 3, 2, 128), np.float32)
    for ti, (fs, k) in enumerate(TAPS):
        w = np.asarray(inputs[f"conv_w{fs}"], np.float32)
        for p in range(128):
            wmain[p, ti, 0, 0:NF] = w[:, 2 * p, k]
            wmain[p, ti, 1, 0:NF] = w[:, 2 * p + 1, k]
    for fs in FSIZES:
        w = np.asarray(inputs[f"conv_w{fs}"], np.float32)
        for j in range(fs):
            for q in range(22):
                wleft[22 * j + q, FSI[fs], 0, 0:NF] = w[:, 256 + 2 * q, j]
                wleft[22 * j + q, FSI[fs], 1, 0:NF] = w[:, 256 + 2 * q + 1, j]

    convb = np.stack(
        [np.asarray(inputs[f"conv_b{fs}"], np.float32) for fs in FSIZES], axis=1
    )

    perm = np.concatenate(
        [np.arange(0, 256), np.arange(256, 512), np.arange(768, 1024), np.arange(512, 768)]
    )  # i,f,g,o -> i,f,o,g

    wih_h = np.zeros((NF, 3, 2, 1024), np.float32)
    bihp_h = np.zeros((128, 2, 8), np.float32)
    whh_h = np.zeros((128, 2, 2, 8, 128), np.float32)
    for d, tag in ((0, "f"), (1, "r")):
        wihm = np.asarray(inputs[f"w_ih_{tag}"], np.float32)[perm]
        whhm = np.asarray(inputs[f"w_hh_{tag}"], np.float32)[perm]
        bsum = (np.asarray(inputs[f"b_ih_{tag}"], np.float32)
                + np.asarray(inputs[f"b_hh_{tag}"], np.float32))[perm]
        for kk in range(3):
            wih_h[:, kk, d, :] = wihm[:, kk * NF : (kk + 1) * NF].T
        bihp_h[:, d, :] = bsum.reshape(8, 128).T
        whh_h[:, d] = whhm.reshape(8, 128, 2, 128).transpose(3, 2, 0, 1)

    headw = np.asarray(inputs["head_w"], np.float32)
    headw_h = headw.T.reshape(4, 128, OUT).transpose(1, 0, 2)
    headb_h = np.asarray(inputs["head_b"], np.float32).reshape(OUT, 1)

    return {
        "emb_p": emb_p,
        "iden": np.eye(128, dtype=BF16),
        "wmain": np.ascontiguousarray(wmain.astype(F8)),
        "wleft": np.ascontiguousarray(wleft.astype(F8)),
        "convb": np.ascontiguousarray(convb),
        "wih": np.ascontiguousarray(wih_h.astype(BF16)),
        "bihp": np.ascontiguousarray(bihp_h),
        "whh": np.ascontiguousarray(whh_h.astype(F8)),
        "headw": np.ascontiguousarray(headw_h.astype(F8)),
        "headb": headb_h,
    }


def prep_core_idx(dialogue, core):
    """(t, b)-ordered token stream; token c*128+p at [p, c]."""
    dia = np.asarray(dialogue[B_CORE * core : B_CORE * (core + 1)], np.int32)
    ids = dia.transpose(1, 0, 2).reshape(-1)  # (t, b, l)
    return np.ascontiguousarray(ids.reshape(NTOK // 128, 128).T)


def kernel(**inputs):
    from concourse.bass_utils import run_bass_kernel_spmd

    nc = get_program()
    shared = prep_shared(inputs)
    dialogue = np.asarray(inputs["dialogue"])
    in_maps = []
    for core in range(NCORES):
        m = dict(shared)
        m["idx_w"] = prep_core_idx(dialogue, core)
        in_maps.append(m)
    res = run_bass_kernel_spmd(nc, in_maps, list(range(NCORES)))
    out = np.zeros((B, T, OUT), np.float32)
    for core in range(NCORES):
        o = res.results[core]["out_t"]  # [32, 256] col = b*64 + t
        out[B_CORE * core : B_CORE * (core + 1)] = o.reshape(OUT, B_CORE, T).transpose(
            1, 2, 0
        )
    return out
